# revision 1
# baseline (speedup 1.0000x reference)
"""Trainium2 Bass kernel for PointNet++-style ball query (nn_BallQuery).

Problem: query [4, 2048, 3] f32, key [4, 8192, 3] f32 -> out [4, 2048, 64] int32.
For each query point, the indices of the first 64 key points (in key order)
with squared distance < 0.1^2; empty slots padded with the first neighbor
index (0 if none).

Sharding (8 NeuronCores): data-parallel over batch B=4 (2 cores per batch),
queries split in halves of 1024 per core; keys of the batch replicated.

Per-core pipeline (8 tiles of 128 queries x 8192 keys, scatters paired):
  PE   : psum = |k|^2 - 2 q.k  via bf16x3-split 21-row contraction
         (exact bf16 products, fp32 accumulate; ~1e-7 accuracy)
  ACT  : sgn  = Sign(psum + (|q|^2-r^2))   (per-partition fp32 bias)
  DVE  : idx  = select(within & rank<=64, rank+C1, -1024)  (one fused custom
         op: compare + inclusive scan + mask; C1 = -1 / +63 for pair halves)
  Pool : out16[slot] = j via local_scatter over a 2-tile pair
  DVE  : pad empty (0) slots with the first neighbor; cast int32
"""

import numpy as np
from contextlib import ExitStack

RADIUS2 = float(np.float32(np.float32(0.1) ** 2))
B, N1, N2, K = 4, 2048, 8192, 64
NCORES = 8
QSHARD = N1 // 2  # 1024 queries per core

_CACHE = {}


# --------------------------------------------------------------------------
# custom DVE op registration
# --------------------------------------------------------------------------

def _register_ballq_op():
    import concourse.dve_ops as dvo
    from concourse.dve_spec import (
        Spec, Src0, Zero, C0, C1, C2, AluOp, scan, select, Bin, lower,
        _has_src1 as has_src1,
    )
    from concourse.dve_uop import DveOpSpec

    name = "BALLQ_IDX"
    if name in dvo._SUB_OPCODE_FOR_NAME:
        return (next(op for op in dvo.OPS if op.name == name),
                next(op for op in dvo.OPS if op.name == "BALLQ_IOTA"))

    w = Bin(AluOp.IS_LT, Src0, Zero)          # sgn < 0  -> within
    s = scan(AluOp.ADD, w)                    # inclusive rank among within
    body = select(w & (s <= C0), s + C1, C2)  # rank<=64 -> rank+C1 else -1024

    def _ref(in0, in1, c0, c1, c2):
        wn = in0 < 0
        sn = np.cumsum(wn, axis=1).astype(np.float32)
        return np.where(wn & (sn <= c0), sn + c1, c2).astype(np.float32)

    spec = Spec(body=body, reference=_ref)
    op = dvo.DveOp(name, spec, subdim=False, uops_sha={}, perf_en={"v3": True})
    dvo.OPS.append(op)
    dvo._SUB_OPCODE_FOR_NAME[name] = dvo._CUSTOM_DVE_ROW_BASE + len(dvo.OPS) - 1
    dvo.CUSTOM_DVE_SPECS[name] = spec
    from concourse.dve_spec import Idx
    spec2 = Spec(body=Idx + Src0 * Zero, reference=lambda in0, in1, c0, c1, c2:
                 np.broadcast_to(np.arange(in0.shape[1], dtype=np.float32),
                                 in0.shape).astype(np.float32))
    op2 = dvo.DveOp("BALLQ_IOTA", spec2, subdim=False, uops_sha={})
    dvo.OPS.append(op2)
    dvo._SUB_OPCODE_FOR_NAME["BALLQ_IOTA"] = dvo._CUSTOM_DVE_ROW_BASE + len(dvo.OPS) - 1
    dvo.CUSTOM_DVE_SPECS["BALLQ_IOTA"] = spec2
    for o, sp in ((op, spec), (op2, spec2)):
        for ver in ("v3", "v4"):
            try:
                compiled = DveOpSpec(
                    name=o.name,
                    opcode=dvo.get_dve_sub_opcode(o.name),
                    uops=lower(sp, ver=ver),
                    rd1_en=has_src1(sp),
                )
                o.uops_sha[ver] = compiled.sha(ver)
            except Exception:
                pass
    return op, op2


# --------------------------------------------------------------------------
# TileContext with the exit-drain wait-splitting workaround (this walrus
# build rejects sync waits attached to the CTRL drain instruction)
# --------------------------------------------------------------------------

def _make_tc_class():
    import concourse.tile as tile
    import concourse.mybir as mybir
    from concourse._compat import not_none as _nn
    from concourse.vector_clock import ScopedClock as _ScopedClock

    class SplitDrainTC(tile.TileContext):
        def _drain_and_barrier(self, tick_clock, wait_clock):
            nc = self.nc
            drain_inst = nc.sync.drain()
            wait_clock.add_sem_waits(
                drain_inst.ins, _ScopedClock({None: tick_clock.global_clock})
            )
            si = drain_inst.ins.sync_info
            if si is not None and si.on_wait:
                waits = list(si.on_wait)
                si.on_wait = []
                bb = _nn(nc.cur_bb).bb
                assert bb.instructions[-1] is drain_inst.ins
                bb.instructions.pop()
                for i in range(len(waits)):
                    nop = nc.sync.nop(hint="drain_wait", nofuse=True)
                    nop.ins.sync_info = mybir.SyncInfo(
                        on_wait=waits[i : i + 1], on_update=[]
                    )
                bb.instructions.append(drain_inst.ins)

            nc.all_engine_barrier()
            assert self.sems is not None
            popped = nc._tile_sem_poison_stack.pop()
            assert popped is self._sem_poison
            nc.clear_and_free_semaphores(list(self.sems.allocated().values()))
            nc.all_engine_barrier()

    return SplitDrainTC


# --------------------------------------------------------------------------
# the Bass program (SPMD: identical on all 8 cores)
# --------------------------------------------------------------------------

def _build_program():
    import concourse.bass as bass
    import concourse.bacc as bacc
    import concourse.mybir as mybir

    ballq_op, iota_op = _register_ballq_op()
    SplitDrainTC = _make_tc_class()
    f32 = mybir.dt.float32
    bf16 = mybir.dt.bfloat16
    i16 = mybir.dt.int16
    i32 = mybir.dt.int32

    nc = bacc.Bacc(None, target_bir_lowering=False)
    q_in = nc.declare_dram_parameter("q", [QSHARD, 3], f32, isOutput=False)
    k_in = nc.declare_dram_parameter("k", [N2, 3], f32, isOutput=False)
    out_t = nc.declare_dram_parameter("out", [QSHARD, K], i32, isOutput=True)

    # DRAM bounce for the 12 distinct key rows (bf16x3 splits + |k|^2 splits)
    kd = nc.dram_tensor("kd_bounce", [12, N2], bf16)

    ntiles = QSHARD // 128  # 8

    with SplitDrainTC(nc) as tc, ExitStack() as ctx:
        singles = ctx.enter_context(tc.tile_pool(name="singles", bufs=1))
        kprep = ctx.enter_context(tc.tile_pool(name="kprep", bufs=1))
        lhs_pool = ctx.enter_context(tc.tile_pool(name="lhs", bufs=1))
        qn_pool = ctx.enter_context(tc.tile_pool(name="qn", bufs=1))
        sgn_pool = ctx.enter_context(tc.tile_pool(name="sgn", bufs=2))
        idx_pool = ctx.enter_context(tc.tile_pool(name="idx", bufs=2))
        o16_pool = ctx.enter_context(tc.tile_pool(name="o16", bufs=3))
        fin_pool = ctx.enter_context(tc.tile_pool(name="fin", bufs=4))
        psum_pool = ctx.enter_context(tc.tile_pool(name="psum", bufs=2, space="PSUM"))
        qd_pool = ctx.enter_context(tc.tile_pool(name="qd", bufs=1, space="DRAM"))

        # ---- key prep: bf16x3 splits in natural layout, bounce to rows ----
        # knat[p, a*3+d] = k[64p + a, d]  (partition-major keys)
        knat = kprep.tile([128, 192], f32)
        nc.sync.dma_start(out=knat[:], in_=k_in[:, :].rearrange("(p a) d -> p (a d)", p=128))

        # planar split tiles: [128, 3, 64] (d-plane major) for contiguous bounce
        ka = kprep.tile([128, 192], bf16)
        kaV = ka[:].rearrange("p (d f) -> p f d", d=3)
        nc.vector.tensor_copy(kaV, knat[:].rearrange("p (f d) -> p f d", d=3))
        r1 = kprep.tile([128, 192], f32)
        nc.vector.tensor_sub(r1[:].rearrange("p (f d) -> p f d", d=3), knat[:].rearrange("p (f d) -> p f d", d=3), ka[:].rearrange("p (d f) -> p f d", d=3))
        kb = kprep.tile([128, 192], bf16)
        kbV = kb[:].rearrange("p (d f) -> p f d", d=3)
        nc.vector.tensor_copy(kbV, r1[:].rearrange("p (f d) -> p f d", d=3))
        r2 = kprep.tile([128, 192], f32)
        nc.vector.tensor_sub(r2[:].rearrange("p (f d) -> p f d", d=3), r1[:].rearrange("p (f d) -> p f d", d=3), kb[:].rearrange("p (d f) -> p f d", d=3))
        kc = kprep.tile([128, 192], bf16)
        kcV = kc[:].rearrange("p (d f) -> p f d", d=3)
        nc.vector.tensor_copy(kcV, r2[:].rearrange("p (f d) -> p f d", d=3))

        # |k|^2 (exact fp32 chain) and its bf16x3 split
        sq = kprep.tile([128, 192], f32)
        nc.vector.tensor_mul(sq[:], knat[:], knat[:])
        ksum = kprep.tile([128, 64], f32)
        nc.vector.tensor_reduce(
            ksum[:], sq[:].rearrange("p (a d) -> p a d", d=3),
            axis=mybir.AxisListType.X, op=mybir.AluOpType.add,
        )
        hA = kprep.tile([128, 64], bf16)
        nc.vector.tensor_copy(hA[:], ksum[:])
        hr1 = kprep.tile([128, 64], f32)
        nc.vector.tensor_sub(hr1[:], ksum[:], hA[:])
        hB = kprep.tile([128, 64], bf16)
        nc.vector.tensor_copy(hB[:], hr1[:])
        hr2 = kprep.tile([128, 64], f32)
        nc.vector.tensor_sub(hr2[:], hr1[:], hB[:])
        hC = kprep.tile([128, 64], bf16)
        nc.vector.tensor_copy(hC[:], hr2[:])

        # bounce out: kd rows 0-2=kaXYZ, 3-5=kbXYZ, 6-8=kcXYZ, 9-11=hABC
        # planar tiles: element (p, d, f) -> kd[d, 64p + f]; inner f contiguous
        for rows, t in ((0, ka), (3, kb), (6, kc)):
            nc.sync.dma_start(
                out=kd[rows:rows + 3, :].rearrange("d (p f) -> p d f", p=128),
                in_=t[:].rearrange("p (d f) -> p d f", d=3),
            )
        hAll = kprep.tile([128, 192], bf16)
        nc.vector.tensor_copy(hAll[:, 0:64], hA[:])
        nc.vector.tensor_copy(hAll[:, 64:128], hB[:])
        nc.vector.tensor_copy(hAll[:, 128:192], hC[:])
        nc.scalar.dma_start(
            out=kd[9:12, :].rearrange("d (p f) -> p d f", p=128),
            in_=hAll[:].rearrange("p (d f) -> p d f", d=3),
        )

        # bounce in: rhs rows (with duplicates) from kd
        # rhs: 0-2 ka, 3-5 ka, 6-8 ka, 9-11 kb, 12-14 kb, 15-17 kc, 18-20 h
        rhs = singles.tile([21, N2], bf16)
        for dst, src in ((0, 0), (3, 0), (6, 0), (9, 3), (12, 3), (15, 6), (18, 9)):
            nc.sync.dma_start(out=rhs[dst:dst + 3, :], in_=kd[src:src + 3, :])

        # ---- scatter data: values j then j again (pairs), via DVE Idx ----
        iota2 = singles.tile([128, 2 * N2], i16)
        nc.vector.memset(iota2[:], 0)
        for hf in range(2):
            nc.vector._custom_dve(
                iota_op,
                out=iota2[:, hf * N2:(hf + 1) * N2],
                in0=iota2[:, hf * N2:(hf + 1) * N2],
            )


        # ---- hoisted per-tile query prep (all tiles up front; keeps the
        # tiny DVE ops out of the scatter windows) -----------------------
        lhsTs, nbs = [], []
        for t in range(ntiles):
            qn = qn_pool.tile([128, 3], f32, tag=f"qn{t}")
            nc.sync.dma_start(out=qn[:], in_=q_in[t * 128:(t + 1) * 128, :])

            # bf16x3 split of q -> qall [128, 21]: the 21 lhsT rows as
            # columns, pre-scaled: [-2qa, -2qb, -2qc, -2qa, -2qb, -2qa, 1]
            qall = qn_pool.tile([128, 21], bf16, tag=f"qall{t}")
            nc.vector.tensor_copy(qall[:, 0:3], qn[:])
            qr1 = qn_pool.tile([128, 3], f32, tag=f"qr1{t}")
            nc.vector.tensor_sub(qr1[:], qn[:], qall[:, 0:3])
            nc.vector.tensor_copy(qall[:, 3:6], qr1[:])
            qr2 = qn_pool.tile([128, 3], f32, tag=f"qr2{t}")
            nc.vector.tensor_sub(qr2[:], qr1[:], qall[:, 3:6])
            nc.vector.tensor_copy(qall[:, 6:9], qr2[:])
            nc.vector.tensor_scalar_mul(qall[:, 9:12], qall[:, 0:3], 1.0)
            nc.vector.tensor_scalar_mul(qall[:, 12:15], qall[:, 3:6], 1.0)
            nc.vector.tensor_scalar_mul(qall[:, 15:18], qall[:, 0:3], 1.0)
            nc.vector.tensor_scalar_mul(qall[:, 0:18], qall[:, 0:18], -2.0)
            nc.vector.memset(qall[:, 18:21], 1.0)

            # bias nb = |q|^2 - r^2 (exact fp32 chain)
            qsq3 = qn_pool.tile([128, 3], f32, tag=f"qsq3{t}")
            nc.vector.tensor_mul(qsq3[:], qn[:], qn[:])
            nb = qn_pool.tile([128, 1], f32, tag=f"nb{t}")
            nc.vector.tensor_reduce(
                nb[:], qsq3[:], axis=mybir.AxisListType.X, op=mybir.AluOpType.add
            )
            nc.vector.tensor_scalar_add(nb[:], nb[:], -RADIUS2)
            nbs.append(nb)

            qd = qd_pool.tile([21, 128], bf16, tag=f"qd{t}")
            eng = nc.scalar if t % 2 == 0 else nc.sync
            eng.dma_start(out=qd[:].rearrange("r p -> p r"), in_=qall[:])
            lhsT = lhs_pool.tile([21, 128], bf16, tag=f"lhsT{t}")
            eng.dma_start(out=lhsT[:], in_=qd[:])
            lhsTs.append(lhsT)

        for pair in range(ntiles // 2):
            idx16 = idx_pool.tile([128, 2 * N2], i16)
            for half in range(2):
                t = pair * 2 + half
                lhsT = lhsTs[t]
                nb = nbs[t]

                # ---- matmuls + sign -------------------------------------
                sgn = sgn_pool.tile([128, N2], bf16, tag="sgn")
                for quarter in range(4):
                    psum = psum_pool.tile([128, 2048], f32, tag="psum")
                    for m in range(4):
                        c0 = quarter * 2048 + m * 512
                        nc.tensor.matmul(
                            psum[:, m * 512:(m + 1) * 512],
                            lhsT[:],
                            rhs[:, c0:c0 + 512],
                            start=True,
                            stop=True,
                        )
                    nc.scalar.activation(
                        out=sgn[:, quarter * 2048:(quarter + 1) * 2048],
                        in_=psum[:],
                        func=mybir.ActivationFunctionType.Sign,
                        bias=nb[:],
                        scale=1.0,
                    )

                # ---- fused compare+scan+mask -> int16 slots -------------
                # even half -> slots 0..63 (C1=-1); odd half -> 64..127 (C1=+63)
                nc.vector._custom_dve(
                    ballq_op,
                    out=idx16[:, half * N2:(half + 1) * N2],
                    in0=sgn[:],
                    s0=float(K),
                    s1=(-1.0 if half == 0 else 63.0),
                    imm2=-1024.0,
                )

            # ---- one scatter per pair of tiles --------------------------
            out16 = o16_pool.tile([128, 2 * K], i16)
            nc.gpsimd.local_scatter(
                out_ap=out16[:],
                data_ap=iota2[:],
                idxs_ap=idx16[:],
                channels=128,
                num_elems=2 * K,
                num_idxs=2 * N2,
            )

            # ---- pad + cast + store per half ----------------------------
            for half in range(2):
                t = pair * 2 + half
                sl = slice(half * K, (half + 1) * K)
                m01 = fin_pool.tile([128, K], mybir.dt.int8, tag="m01")
                nc.vector.tensor_scalar(
                    out=m01[:], in0=out16[:, sl], scalar1=0.0, scalar2=None,
                    op0=mybir.AluOpType.is_gt,
                )
                final = fin_pool.tile([128, K], i32, tag="final")
                nc.vector.select(
                    out=final[:],
                    mask=m01[:],
                    on_true=out16[:, sl],
                    on_false=out16[:, half * K:half * K + 1].to_broadcast([128, K]),
                )
                nc.sync.dma_start(out=out_t[t * 128:(t + 1) * 128, :], in_=final[:])

    nc.finalize()
    return nc


def _get_program():
    if "nc" not in _CACHE:
        _CACHE["nc"] = _build_program()
    return _CACHE["nc"]


# --------------------------------------------------------------------------
# public entry point
# --------------------------------------------------------------------------

def kernel(query: np.ndarray, key: np.ndarray) -> np.ndarray:
    from concourse.bass_utils import run_bass_kernel_spmd

    query = np.ascontiguousarray(np.asarray(query, dtype=np.float32))
    key = np.ascontiguousarray(np.asarray(key, dtype=np.float32))
    assert query.shape == (B, N1, 3) and key.shape == (B, N2, 3)

    nc = _get_program()

    in_maps = []
    for core in range(NCORES):
        b = core // 2
        h = core % 2
        in_maps.append({
            "q": np.ascontiguousarray(query[b, h * QSHARD:(h + 1) * QSHARD]),
            "k": np.ascontiguousarray(key[b]),
        })

    res = run_bass_kernel_spmd(nc, in_maps, core_ids=list(range(NCORES)))

    out = np.empty((B, N1, K), dtype=np.int32)
    for core in range(NCORES):
        b = core // 2
        h = core % 2
        out[b, h * QSHARD:(h + 1) * QSHARD] = res.results[core]["out"]
    return out



# revision 5
# speedup vs baseline: 1.7451x; 1.7451x over previous
"""Trainium2 Bass kernel for PointNet++-style ball query (nn_BallQuery).

Problem: query [4, 2048, 3] f32, key [4, 8192, 3] f32 -> out [4, 2048, 64] int32.
For each query point, the indices of the first 64 key points (in key order)
with squared distance < 0.1^2; empty slots padded with the first neighbor
index (0 if none).

Sharding (8 NeuronCores): data-parallel over batch B=4 (2 cores per batch),
queries split in halves of 1024 per core; keys of the batch replicated.

Per-core pipeline (8 tiles of 128 queries x 8192 keys):
  PE   : psum = |k|^2 - 2 q.k  via bf16x3-split 21-row contraction
  ACT  : sgn  = Sign(psum + (|q|^2-r^2))   (per-partition fp32 bias)
  DVE  : idx  = select(within & rank<=64, rank-1, -1024)  (fused custom op)
  Pool : out16[rank-1] = j  via per-tile local_scatter (8192 idxs)
  DVE  : pad empty slots with first neighbor + cast int32 (fused custom op)

The -2 scaling lives on the key side (rhs rows -2ka/-2kb/-2kc); the query
side lhsT rows are the raw bf16x3 splits assembled via small SBUF->SBUF
DMAs from a host-transposed qT input (no DRAM bounce).
"""

import numpy as np
from contextlib import ExitStack

RADIUS2 = float(np.float32(np.float32(0.1) ** 2))
B, N1, N2, K = 4, 2048, 8192, 64
NCORES = 8
QSHARD = N1 // 2  # 1024 queries per core
NTILES = QSHARD // 128  # 8

_CACHE = {}


# --------------------------------------------------------------------------
# custom DVE op registration
# --------------------------------------------------------------------------

def _register_ballq_ops():
    import concourse.dve_ops as dvo
    from concourse.dve_spec import (
        Spec, Src0, Zero, C0, C1, C2, AluOp, scan, select, Bin, lower, Idx,
        _has_src1 as has_src1,
    )
    from concourse.dve_uop import DveOpSpec

    if "BALLQ_IDX" in dvo._SUB_OPCODE_FOR_NAME:
        ops = {op.name: op for op in dvo.OPS}
        return ops["BALLQ_IDX"], ops["BALLQ_IOTA"], ops["BALLQ_PAD"]

    # BALLQ_IDX: within = sgn < 0; rank = cumsum(within);
    # out = rank-1 if within & rank <= 64 else -1024
    w = Bin(AluOp.IS_LT, Src0, Zero)
    s = scan(AluOp.ADD, w)
    body = select(w & (s <= C0), s + C1, C2)

    def _ref_idx(in0, in1, c0, c1, c2):
        wn = in0 < 0
        sn = np.cumsum(wn, axis=1).astype(np.float32)
        return np.where(wn & (sn <= c0), sn + c1, c2).astype(np.float32)

    spec_idx = Spec(body=body, reference=_ref_idx)

    # BALLQ_IOTA: out[p, j] = j  (in0 is read but multiplied by 0, so an
    # uninitialized int16 buffer is safe)
    spec_iota = Spec(
        body=Idx + Src0 * Zero,
        reference=lambda in0, in1, c0, c1, c2: np.broadcast_to(
            np.arange(in0.shape[1], dtype=np.float32), in0.shape
        ).astype(np.float32),
    )

    # BALLQ_PAD: out = in0 if in0 > 0 else s0 (per-partition first-neighbor)
    spec_pad = Spec(
        body=select(Src0 > Zero, Src0, C0),
        reference=lambda in0, in1, c0, c1, c2: np.where(
            in0 > 0, in0, c0
        ).astype(np.float32),
    )

    out_ops = []
    for name, sp in (("BALLQ_IDX", spec_idx), ("BALLQ_IOTA", spec_iota),
                     ("BALLQ_PAD", spec_pad)):
        op = dvo.DveOp(name, sp, subdim=False, uops_sha={})
        dvo.OPS.append(op)
        dvo._SUB_OPCODE_FOR_NAME[name] = dvo._CUSTOM_DVE_ROW_BASE + len(dvo.OPS) - 1
        dvo.CUSTOM_DVE_SPECS[name] = sp
        for ver in ("v3", "v4"):
            try:
                compiled = DveOpSpec(
                    name=op.name,
                    opcode=dvo.get_dve_sub_opcode(op.name),
                    uops=lower(sp, ver=ver),
                    rd1_en=has_src1(sp),
                )
                op.uops_sha[ver] = compiled.sha(ver)
            except Exception:
                pass
        out_ops.append(op)
    return tuple(out_ops)


# --------------------------------------------------------------------------
# TileContext with the exit-drain wait-splitting workaround (this walrus
# build rejects sync waits attached to the CTRL drain instruction)
# --------------------------------------------------------------------------

def _make_tc_class():
    import concourse.tile as tile
    import concourse.mybir as mybir
    from concourse._compat import not_none as _nn
    from concourse.vector_clock import ScopedClock as _ScopedClock

    class SplitDrainTC(tile.TileContext):
        def _drain_and_barrier(self, tick_clock, wait_clock):
            nc = self.nc
            drain_inst = nc.sync.drain()
            wait_clock.add_sem_waits(
                drain_inst.ins, _ScopedClock({None: tick_clock.global_clock})
            )
            si = drain_inst.ins.sync_info
            if si is not None and si.on_wait:
                waits = list(si.on_wait)
                si.on_wait = []
                bb = _nn(nc.cur_bb).bb
                assert bb.instructions[-1] is drain_inst.ins
                bb.instructions.pop()
                for i in range(len(waits)):
                    nop = nc.sync.nop(hint="drain_wait", nofuse=True)
                    nop.ins.sync_info = mybir.SyncInfo(
                        on_wait=waits[i : i + 1], on_update=[]
                    )
                bb.instructions.append(drain_inst.ins)

            nc.all_engine_barrier()
            assert self.sems is not None
            popped = nc._tile_sem_poison_stack.pop()
            assert popped is self._sem_poison
            nc.clear_and_free_semaphores(list(self.sems.allocated().values()))
            nc.all_engine_barrier()

    return SplitDrainTC


# --------------------------------------------------------------------------
# the Bass program (SPMD: identical on all 8 cores)
# --------------------------------------------------------------------------

def _build_program():
    import concourse.bass as bass
    import concourse.bacc as bacc
    import concourse.mybir as mybir

    idx_op, iota_op, pad_op = _register_ballq_ops()
    SplitDrainTC = _make_tc_class()
    f32 = mybir.dt.float32
    bf16 = mybir.dt.bfloat16
    i16 = mybir.dt.int16
    i32 = mybir.dt.int32

    nc = bacc.Bacc(None, target_bir_lowering=False)
    q_in = nc.declare_dram_parameter("q", [QSHARD, 3], f32, isOutput=False)
    qT_in = nc.declare_dram_parameter("qT", [3, QSHARD], f32, isOutput=False)
    k_in = nc.declare_dram_parameter("k", [N2, 3], f32, isOutput=False)
    out_t = nc.declare_dram_parameter("out", [QSHARD, K], i32, isOutput=True)

    # DRAM bounce for the 12 distinct key rows (-2*bf16x3 splits + |k|^2 splits)
    kd = nc.dram_tensor("kd_bounce", [12, N2], bf16)

    with SplitDrainTC(nc) as tc, ExitStack() as ctx:
        singles = ctx.enter_context(tc.tile_pool(name="singles", bufs=1))
        kprep = ctx.enter_context(tc.tile_pool(name="kprep", bufs=1))
        qprep = ctx.enter_context(tc.tile_pool(name="qprep", bufs=1))
        sgn_pool = ctx.enter_context(tc.tile_pool(name="sgn", bufs=3))
        idx_pool = ctx.enter_context(tc.tile_pool(name="idx", bufs=2))
        o16_pool = ctx.enter_context(tc.tile_pool(name="o16", bufs=4))
        fin_pool = ctx.enter_context(tc.tile_pool(name="fin", bufs=4))
        psum_pool = ctx.enter_context(tc.tile_pool(name="psum", bufs=2, space="PSUM"))

        # ---- input loads (SP queue) -------------------------------------
        # knat[p, a*3+d] = k[64p + a, d]  (partition-major keys)
        knat = kprep.tile([128, 192], f32)
        nc.sync.dma_start(out=knat[:], in_=k_in[:, :].rearrange("(p a) d -> p (a d)", p=128))
        # qT rows x/y/z on partitions 0-2
        qTt = qprep.tile([3, QSHARD], f32)
        nc.sync.dma_start(out=qTt[:], in_=qT_in[:, :])
        # q natural for the bias: qn[p, t*3+d] = q[t*128+p, d]
        qn = qprep.tile([128, NTILES * 3], f32)
        nc.sync.dma_start(
            out=qn[:].rearrange("p (t d) -> p t d", d=3),
            in_=q_in[:, :].rearrange("(t p) d -> p t d", p=128),
        )

        # ---- key prep on [128, 192] tiles (DVE) -------------------------
        ka = kprep.tile([128, 192], bf16)
        kaV = ka[:].rearrange("p (d f) -> p f d", d=3)
        nc.vector.tensor_copy(kaV, knat[:].rearrange("p (f d) -> p f d", d=3))
        r1 = kprep.tile([128, 192], f32)
        nc.vector.tensor_sub(r1[:].rearrange("p (f d) -> p f d", d=3),
                             knat[:].rearrange("p (f d) -> p f d", d=3),
                             ka[:].rearrange("p (d f) -> p f d", d=3))
        kb = kprep.tile([128, 192], bf16)
        kbV = kb[:].rearrange("p (d f) -> p f d", d=3)
        nc.vector.tensor_copy(kbV, r1[:].rearrange("p (f d) -> p f d", d=3))
        r2 = kprep.tile([128, 192], f32)
        nc.vector.tensor_sub(r2[:].rearrange("p (f d) -> p f d", d=3),
                             r1[:].rearrange("p (f d) -> p f d", d=3),
                             kb[:].rearrange("p (d f) -> p f d", d=3))
        kc = kprep.tile([128, 192], bf16)
        kcV = kc[:].rearrange("p (d f) -> p f d", d=3)
        nc.vector.tensor_copy(kcV, r2[:].rearrange("p (f d) -> p f d", d=3))
        # fold the -2 of -2*q.k into the key side (exact in bf16)
        nc.vector.tensor_scalar_mul(ka[:], ka[:], -2.0)
        nc.vector.tensor_scalar_mul(kb[:], kb[:], -2.0)
        nc.vector.tensor_scalar_mul(kc[:], kc[:], -2.0)

        # |k|^2 (exact fp32 chain) and its bf16x3 split
        sq = kprep.tile([128, 192], f32)
        nc.vector.tensor_mul(sq[:], knat[:], knat[:])
        ksum = kprep.tile([128, 64], f32)
        nc.vector.tensor_reduce(
            ksum[:], sq[:].rearrange("p (a d) -> p a d", d=3),
            axis=mybir.AxisListType.X, op=mybir.AluOpType.add,
        )
        hA = kprep.tile([128, 64], bf16)
        nc.vector.tensor_copy(hA[:], ksum[:])
        hr1 = kprep.tile([128, 64], f32)
        nc.vector.tensor_sub(hr1[:], ksum[:], hA[:])
        hB = kprep.tile([128, 64], bf16)
        nc.vector.tensor_copy(hB[:], hr1[:])
        hr2 = kprep.tile([128, 64], f32)
        nc.vector.tensor_sub(hr2[:], hr1[:], hB[:])
        hC = kprep.tile([128, 64], bf16)
        nc.vector.tensor_copy(hC[:], hr2[:])
        hAll = kprep.tile([128, 192], bf16)
        nc.vector.tensor_copy(hAll[:, 0:64], hA[:])
        nc.vector.tensor_copy(hAll[:, 64:128], hB[:])
        nc.vector.tensor_copy(hAll[:, 128:192], hC[:])

        # bounce out: kd rows 0-2=-2kaXYZ, 3-5=-2kbXYZ, 6-8=-2kcXYZ, 9-11=hABC
        for rows, t in ((0, ka), (3, kb), (6, kc)):
            nc.sync.dma_start(
                out=kd[rows:rows + 3, :].rearrange("d (p f) -> p d f", p=128),
                in_=t[:].rearrange("p (d f) -> p d f", d=3),
            )
        nc.sync.dma_start(
            out=kd[9:12, :].rearrange("d (p f) -> p d f", p=128),
            in_=hAll[:].rearrange("p (d f) -> p d f", d=3),
        )

        # bounce in: rhs rows (with duplicates) from kd
        # rhs: 0-2 -2ka, 3-5 -2ka, 6-8 -2ka, 9-11 -2kb, 12-14 -2kb, 15-17 -2kc, 18-20 h
        rhs = singles.tile([21, N2], bf16)
        for dst, src in ((0, 0), (3, 0), (6, 0), (9, 3), (12, 3), (15, 6), (18, 9)):
            nc.scalar.dma_start(out=rhs[dst:dst + 3, :], in_=kd[src:src + 3, :])

        # ---- query prep: bf16x3 split on [3, QSHARD], assemble lhsT -----
        qa = qprep.tile([3, QSHARD], bf16)
        nc.vector.tensor_copy(qa[:], qTt[:])
        qr1 = qprep.tile([3, QSHARD], f32)
        nc.vector.tensor_sub(qr1[:], qTt[:], qa[:])
        qb = qprep.tile([3, QSHARD], bf16)
        nc.vector.tensor_copy(qb[:], qr1[:])
        qr2 = qprep.tile([3, QSHARD], f32)
        nc.vector.tensor_sub(qr2[:], qr1[:], qb[:])
        qc = qprep.tile([3, QSHARD], bf16)
        nc.vector.tensor_copy(qc[:], qr2[:])

        # lhsT rows: 0-2 qa, 3-5 qb, 6-8 qc, 9-11 qa, 12-14 qb, 15-17 qa, 18-20 ones
        lhsT = singles.tile([21, QSHARD], bf16)
        nc.vector.memset(lhsT[:], 1.0)  # rows 18-20 stay 1.0 (ones rows)
        for dst, src in ((0, qa), (3, qb), (6, qc), (9, qa), (12, qb), (15, qa)):
            nc.sync.dma_start(out=lhsT[dst:dst + 3, :], in_=src[:])

        # bias nb = |q|^2 - r^2 per tile (exact fp32 chain)
        qsq = qprep.tile([128, NTILES * 3], f32)
        nc.vector.tensor_mul(qsq[:], qn[:], qn[:])
        nb_all = qprep.tile([128, NTILES], f32)
        nc.vector.tensor_reduce(
            nb_all[:], qsq[:].rearrange("p (t d) -> p t d", d=3),
            axis=mybir.AxisListType.X, op=mybir.AluOpType.add,
        )
        nc.vector.tensor_scalar_add(nb_all[:], nb_all[:], -RADIUS2)

        # ---- scatter data: iota over an uninitialized buffer ------------
        iota = singles.tile([128, N2], i16)
        nc.vector._custom_dve(iota_op, out=iota[:], in0=iota[:])

        # ---- steady state: per-tile mm -> sign -> scan -> scatter -------
        sgns, idxs, outs16, finals = [], [], [], []
        for t in range(NTILES):
            sgn = sgn_pool.tile([128, N2], bf16, tag="sgn")
            for quarter in range(4):
                psum = psum_pool.tile([128, 2048], f32, tag="psum")
                for m in range(4):
                    c0 = quarter * 2048 + m * 512
                    nc.tensor.matmul(
                        psum[:, m * 512:(m + 1) * 512],
                        lhsT[:, t * 128:(t + 1) * 128],
                        rhs[:, c0:c0 + 512],
                        start=True,
                        stop=True,
                    )
                nc.scalar.activation(
                    out=sgn[:, quarter * 2048:(quarter + 1) * 2048],
                    in_=psum[:],
                    func=mybir.ActivationFunctionType.Sign,
                    bias=nb_all[:, t:t + 1],
                    scale=1.0,
                )
            sgns.append(sgn)

            # DVE scan: slots 0..63 or -1024
            idx16 = idx_pool.tile([128, N2], i16, tag="idx")
            nc.vector._custom_dve(
                idx_op, out=idx16[:], in0=sgn[:],
                s0=float(K), s1=-1.0, imm2=-1024.0,
            )
            idxs.append(idx16)

            # GPSIMD scatter (per tile)
            out16 = o16_pool.tile([128, K], i16, tag="o16")
            nc.gpsimd.local_scatter(
                out_ap=out16[:],
                data_ap=iota[:],
                idxs_ap=idx16[:],
                channels=128,
                num_elems=K,
                num_idxs=N2,
            )
            outs16.append(out16)

            # pad+store for older tiles, emitted late so the DVE queue head
            # never blocks on a still-running scatter
            if t >= 2:
                _emit_pad_store(nc, pad_op, fin_pool, outs16, finals, out_t, t - 2, i32)

        for tt in (NTILES - 2, NTILES - 1):
            _emit_pad_store(nc, pad_op, fin_pool, outs16, finals, out_t, tt, i32)

    nc.finalize()
    return nc


def _emit_pad_store(nc, pad_op, fin_pool, outs16, finals, out_t, t, i32):
    import concourse.mybir as mybir

    out16 = outs16[t]
    first = fin_pool.tile([128, 1], mybir.dt.float32, tag="first")
    nc.vector.tensor_copy(first[:], out16[:, 0:1])
    final = fin_pool.tile([128, 64], i32, tag="final")
    nc.vector._custom_dve(
        pad_op, out=final[:], in0=out16[:], s0=first[:],
    )
    finals.append(final)
    nc.sync.dma_start(out=out_t[t * 128:(t + 1) * 128, :], in_=final[:])


def _get_program():
    if "nc" not in _CACHE:
        _CACHE["nc"] = _build_program()
    return _CACHE["nc"]


def _in_maps(query: np.ndarray, key: np.ndarray):
    in_maps = []
    for core in range(NCORES):
        b = core // 2
        h = core % 2
        qs = np.ascontiguousarray(query[b, h * QSHARD:(h + 1) * QSHARD])
        in_maps.append({
            "q": qs,
            "qT": np.ascontiguousarray(qs.T),
            "k": np.ascontiguousarray(key[b]),
        })
    return in_maps


# --------------------------------------------------------------------------
# public entry point
# --------------------------------------------------------------------------

def kernel(query: np.ndarray, key: np.ndarray) -> np.ndarray:
    from concourse.bass_utils import run_bass_kernel_spmd

    query = np.ascontiguousarray(np.asarray(query, dtype=np.float32))
    key = np.ascontiguousarray(np.asarray(key, dtype=np.float32))
    assert query.shape == (B, N1, 3) and key.shape == (B, N2, 3)

    nc = _get_program()
    res = run_bass_kernel_spmd(nc, _in_maps(query, key), core_ids=list(range(NCORES)))

    out = np.empty((B, N1, K), dtype=np.int32)
    for core in range(NCORES):
        b = core // 2
        h = core % 2
        out[b, h * QSHARD:(h + 1) * QSHARD] = res.results[core]["out"]
    return out
